# revision 4
# baseline (speedup 1.0000x reference)
"""Trainium2 Bass kernel for DiscreteKeyValueBottleneck (VQ codebook lookup).

Problem: x [4,1024,768] -> per head h (12): tokens xh [4096, 64];
argmin_c ||xh - key_embed[h,c]||^2 over c in [0,4096); gather values[h, ind]
-> out [4,1024,768].

Sharding: data-parallel over tokens (4096/8 = 512 tokens per core),
codebooks replicated; no collectives.

Per core, per block (head, 128-token tile):
  - Scoring s = xh @ ke^T - |ke|^2/2 (argmax of s == argmin of dist).
    bf16-split precision: s = x_hi@e_hi + x_lo@e_hi + [x_hi;1;1]@[e_lo;e2a;e2b]
    (error ~1e-7; validated 3 argmax flips of 49152 vs fp64).
    Streams run tok-major: psum [tok=128, c-chunk=1024], K on partitions.
  - Per 1024-c chunk: VectorE reduce_max -> chunk max m4[:, ch]; onehot
    chunk = is_ge(s, m4[ch]) (per-partition scalar operand) directly from
    PSUM -> bf16 {0,1}; PSUM freed immediately (pipelines with next chunk).
    Optionally on ScalarE instead: d = Identity(s - m4) then exp(2.7e9*d)
    (winner is bitwise-equal to the max -> d==0 -> weight exactly 1).
  - onehot tok-major -> c-major via DMA xbar transpose (idle DMA engines;
    out[:, ci, :] = c-chunk ci with c = ci*128 + partition).
  - Gather: per 128-c chunk matmul g_part[:, j] += ohT_ci^T @ vals_ci,
    accumulated per 1024-chunk j; combine with chunk-winner flags
    (flag_j = is_ge(m4_j, max_j m4_j)): out = sum_j flag_j * g_part_j.
"""

import sys

sys.path.insert(0, "/opt/trn_rl_repo")

import numpy as np
import ml_dtypes

BF16 = ml_dtypes.bfloat16

B, T, H, C, DK, DV = 4, 1024, 12, 4096, 64, 64
DIM = H * DK
NCORES = 8
NTOK = B * T            # 4096 tokens total
TOK = NTOK // NCORES    # 512 tokens per core
NTT = TOK // 128        # 4 tok-tiles per core
NCH = 4                 # psum score chunks per block (C/NCH = 1024 c each)
CCHUNK = C // NCH       # 1024
NCC = C // 128          # 32 c-chunks of 128 (gather granularity)
NCPC = NCC // NCH       # 8 gather chunks per score chunk

_CACHE = {}


def _build(act_every=0):
    """act_every=k: every k-th block's onehot runs on ScalarE (0: never)."""
    from contextlib import ExitStack
    from concourse import bass, tile
    import concourse.mybir as mybir
    import waitfix

    fp32 = mybir.dt.float32
    bf16 = mybir.dt.bfloat16

    nc = bass.Bass("TRN2")

    xhi = nc.declare_dram_parameter("xhi", [DK, H, TOK], bf16, isOutput=False)
    xlo = nc.declare_dram_parameter("xlo", [DK, H, TOK], bf16, isOutput=False)
    xb = nc.declare_dram_parameter("xb", [DK + 2, H, TOK], bf16, isOutput=False)
    keh = nc.declare_dram_parameter("keh", [H, DK, C], bf16, isOutput=False)
    keb = nc.declare_dram_parameter("keb", [H, DK + 2, C], bf16, isOutput=False)
    vals = nc.declare_dram_parameter("vals", [H, 128, NCC * DV], bf16, isOutput=False)
    out = nc.declare_dram_parameter("out", [TOK, H * DV], fp32, isOutput=True)

    with tile.TileContext(nc) as tc:
        with ExitStack() as ctx:
            x_pool = ctx.enter_context(tc.tile_pool(name="xp", bufs=1))
            ke_pool = ctx.enter_context(tc.tile_pool(name="kep", bufs=3))
            oh_pool = ctx.enter_context(tc.tile_pool(name="ohp", bufs=3))
            oht_pool = ctx.enter_context(tc.tile_pool(name="ohtp", bufs=3))
            m_pool = ctx.enter_context(tc.tile_pool(name="mp", bufs=6))
            d_pool = ctx.enter_context(tc.tile_pool(name="dp", bufs=2))
            acc_pool = ctx.enter_context(tc.tile_pool(name="accp", bufs=3))
            out_pool = ctx.enter_context(tc.tile_pool(name="outp", bufs=1))
            ps_pool = ctx.enter_context(tc.tile_pool(name="psc", bufs=3, space="PSUM"))
            pg_pool = ctx.enter_context(tc.tile_pool(name="psg", bufs=2, space="PSUM"))

            # resident x tiles
            xhi_sb = x_pool.tile([DK, H * TOK], bf16)
            nc.sync.dma_start(xhi_sb[:], xhi[:].rearrange("k h t -> k (h t)"))
            xlo_sb = x_pool.tile([DK, H * TOK], bf16)
            nc.sync.dma_start(xlo_sb[:], xlo[:].rearrange("k h t -> k (h t)"))
            xb_sb = x_pool.tile([DK + 2, H * TOK], bf16)
            nc.sync.dma_start(xb_sb[:], xb[:].rearrange("k h t -> k (h t)"))

            out_sb = out_pool.tile([128, NTT * H * DV], fp32)

            blk = 0
            for h in range(H):
                keh_t = ke_pool.tile([DK, C], bf16, tag="keh")
                nc.sync.dma_start(keh_t[:], keh[h])
                keb_t = ke_pool.tile([DK + 2, C], bf16, tag="keb")
                nc.sync.dma_start(keb_t[:], keb[h])
                vals_t = ke_pool.tile([128, NCC * DV], bf16, tag="vals")
                nc.sync.dma_start(vals_t[:], vals[h])

                for tt in range(NTT):
                    tok0 = h * TOK + tt * 128
                    lhs_hi = xhi_sb[:, tok0 : tok0 + 128]
                    lhs_lo = xlo_sb[:, tok0 : tok0 + 128]
                    lhs_b = xb_sb[:, tok0 : tok0 + 128]

                    m4 = m_pool.tile([128, NCH], fp32, tag="m4")
                    oh = oh_pool.tile([128, C], bf16, tag="oh")
                    use_act = act_every and (blk % act_every == act_every - 1)

                    for ch in range(NCH):
                        ps = ps_pool.tile([128, CCHUNK], fp32, tag="sc")
                        for sub in range(CCHUNK // 512):
                            c0 = ch * CCHUNK + sub * 512
                            po = ps[:, sub * 512 : sub * 512 + 512]
                            nc.tensor.matmul(
                                po, lhs_hi, keh_t[:, c0 : c0 + 512],
                                start=True, stop=False,
                            )
                            nc.tensor.matmul(
                                po, lhs_lo, keh_t[:, c0 : c0 + 512],
                                start=False, stop=False,
                            )
                            nc.tensor.matmul(
                                po, lhs_b, keb_t[:, c0 : c0 + 512],
                                start=False, stop=True,
                            )
                        nc.vector.reduce_max(
                            m4[:, ch : ch + 1], ps[:], axis=mybir.AxisListType.X
                        )
                        ohc = oh[:, ch * CCHUNK : (ch + 1) * CCHUNK]
                        if use_act:
                            d_sb = d_pool.tile([128, CCHUNK], fp32, tag="d")
                            mneg = m_pool.tile([128, 1], fp32, tag="mneg")
                            nc.vector.tensor_scalar(
                                mneg[:], m4[:, ch : ch + 1], -1.0, None,
                                op0=mybir.AluOpType.mult,
                            )
                            nc.scalar.activation(
                                d_sb[:], ps[:],
                                mybir.ActivationFunctionType.Identity,
                                bias=mneg[:], scale=1.0,
                            )
                            nc.scalar.activation(
                                ohc, d_sb[:],
                                mybir.ActivationFunctionType.Exp,
                                bias=0.0, scale=2.7e9,
                            )
                        else:
                            nc.vector.tensor_scalar(
                                ohc, ps[:], m4[:, ch : ch + 1], None,
                                op0=mybir.AluOpType.is_ge,
                            )

                    # tok-major -> c-major via DMA xbar transpose
                    oht = oht_pool.tile([128, NCC, 128], bf16, tag="oht")
                    nc.sync.dma_start_transpose(oht[:], oh[:])

                    # gather partials per score-chunk
                    g_ps = pg_pool.tile([128, NCH * DV], fp32, tag="g")
                    for ci in range(NCC):
                        j = ci // NCPC
                        nc.tensor.matmul(
                            g_ps[:, j * DV : (j + 1) * DV],
                            oht[:, ci, :],
                            vals_t[:, ci * DV : (ci + 1) * DV],
                            start=(ci % NCPC == 0),
                            stop=(ci % NCPC == NCPC - 1),
                        )

                    # combine partials with chunk-winner flags
                    mg = m_pool.tile([128, 1], fp32, tag="mg")
                    nc.vector.reduce_max(mg[:], m4[:], axis=mybir.AxisListType.X)
                    flags = m_pool.tile([128, NCH], fp32, tag="fl")
                    nc.vector.tensor_scalar(
                        flags[:], m4[:], mg[:], None, op0=mybir.AluOpType.is_ge
                    )
                    ob = out_sb[:, (tt * H + h) * DV : (tt * H + h + 1) * DV]
                    acc = acc_pool.tile([128, DV], fp32, tag="acc")
                    tmp = acc_pool.tile([128, DV], fp32, tag="tmp")
                    nc.vector.tensor_scalar(
                        acc[:], g_ps[:, 0:DV], flags[:, 0:1], None,
                        op0=mybir.AluOpType.mult,
                    )
                    for j in range(1, NCH - 1):
                        nc.vector.tensor_scalar(
                            tmp[:], g_ps[:, j * DV : (j + 1) * DV],
                            flags[:, j : j + 1], None, op0=mybir.AluOpType.mult,
                        )
                        nc.vector.tensor_add(acc[:], acc[:], tmp[:])
                    # last: scalar_tensor_tensor writes out_sb directly
                    nc.vector.scalar_tensor_tensor(
                        ob,
                        g_ps[:, (NCH - 1) * DV : NCH * DV],
                        flags[:, NCH - 1 : NCH],
                        acc[:],
                        op0=mybir.AluOpType.mult,
                        op1=mybir.AluOpType.add,
                    )
                    blk += 1

            nc.sync.dma_start(
                out[:].rearrange("(a p) f -> p a f", p=128),
                out_sb[:].rearrange("p (a f) -> p a f", a=NTT),
            )

    waitfix.strip_redundant_waits(nc)
    return nc


def _host_prep(x, key_embed, values):
    """Builds per-core input maps (numpy, bf16 splits, layouts)."""
    x = np.asarray(x, np.float32)
    ke = np.asarray(key_embed, np.float32)
    vv = np.asarray(values, np.float32)

    xh = x.reshape(B * T, H, DK).transpose(1, 0, 2)  # [H, NTOK, DK]
    x_hi = xh.astype(BF16)
    x_lo = (xh - x_hi.astype(np.float32)).astype(BF16)

    e_hi = ke.astype(BF16)
    e_lo = (ke - e_hi.astype(np.float32)).astype(BF16)
    e2 = -(0.5 * np.einsum("hcd,hcd->hc", ke.astype(np.float64), ke.astype(np.float64)))
    e2 = e2.astype(np.float32)
    e2a = e2.astype(BF16)
    e2b = (e2 - e2a.astype(np.float32)).astype(BF16)

    keh = np.ascontiguousarray(e_hi.transpose(0, 2, 1))
    keb = np.ascontiguousarray(
        np.concatenate(
            [e_lo.transpose(0, 2, 1), e2a[:, None, :], e2b[:, None, :]], axis=1
        ).astype(BF16)
    )
    vh = vv.astype(BF16).reshape(H, NCC, 128, DV).transpose(0, 2, 1, 3)
    vh = np.ascontiguousarray(vh.reshape(H, 128, NCC * DV))

    in_maps = []
    for core in range(NCORES):
        t0 = core * TOK
        a_hi = np.ascontiguousarray(x_hi[:, t0 : t0 + TOK, :].transpose(2, 0, 1))
        a_lo = np.ascontiguousarray(x_lo[:, t0 : t0 + TOK, :].transpose(2, 0, 1))
        a_b = np.ascontiguousarray(
            np.concatenate([a_hi, np.ones((2, H, TOK), BF16)], axis=0)
        )
        in_maps.append(
            {"xhi": a_hi, "xlo": a_lo, "xb": a_b, "keh": keh, "keb": keb, "vals": vh}
        )
    return in_maps


def _get_nc():
    if "nc" not in _CACHE:
        _CACHE["nc"] = _build()
    return _CACHE["nc"]


def run_cores(in_maps, trace=False):
    from concourse.bass_utils import run_bass_kernel_spmd

    nc = _get_nc()
    return run_bass_kernel_spmd(nc, in_maps, list(range(NCORES)), trace=trace)


def kernel(x, mask, key_embed, values, key_optim):
    in_maps = _host_prep(x, key_embed, values)
    res = run_cores(in_maps, trace=False)
    outs = [np.asarray(res.results[c]["out"], np.float32) for c in range(NCORES)]
    full = np.concatenate(outs, axis=0)
    return full.reshape(B, T, H * DV)
